# revision 16
# baseline (speedup 1.0000x reference)
"""Causal self-attention Trainium2 kernel (Bass/Tile, SPMD over 8 NeuronCores).

Problem: B=2, T=2048, C=1024, H=16 heads, HD=64; out = proj(softmax(mask(q k^T * s)) v).

Sharding (Megatron-style): core c handles batch b=c//4 and head group g=c%4
(4 heads).  Each core computes qkv for its heads, causal attention, and a
partial projection out_partial = y_local @ W_proj[rows of g].  Host sums the 4
partials per batch; q/k biases are applied on-device, while the v/proj biases
are folded into a host-side constant correction (softmax rows sum to 1, so
attn @ (v + bv) = attn @ v + bv).

Device layout notes:
  - Host passes x^T so the C-contraction dim lands on SBUF partitions.
  - q,k are produced transposed (qT/kT: [head cols, T]); scores are computed
    transposed (S^T: [keys, queries]) so the AV matmul needs no transposes.
  - v gets an appended ones-column; the AV matmul then yields softmax row sums
    (l) in output row 64, normalized via reciprocal + partition-broadcast.
  - All matmuls run in float32r (~1.6e-4 matmul rel err, 4x faster than fp32).
  - Scores for two key-tiles land in one 2-bank PSUM tensor and share one
    wide exp ACTIVATE (amortizes the scalar engine's per-op overhead).
  - Emission interleaves attention units with qkv/proj matmul chains at fine
    grain so the tensor engine never idles long enough for the HAM clock
    gate to re-throttle it to 1.2 GHz while exp drains on the scalar engine.
"""

import numpy as np

import concourse.bacc as bacc
import concourse.mybir as mybir
import concourse.tile as tile
from concourse.bass_utils import run_bass_kernel_spmd

B, T, C, H, HD = 2, 2048, 1024, 16, 64
SCALE = 0.1 / (HD**0.5)
NCORES = 8
NGROUPS = 4           # head groups (tensor-parallel dim)
HPG = H // NGROUPS    # 4 heads per core
GC = HPG * HD         # 256 head-cols per core
QKC = 2 * GC          # 512 q+k cols per core
TCH = 512             # psum free-dim chunk
NT = T // 128         # 16 row tiles
NQB = T // TCH        # 4 query blocks
NKC = 8               # C/128 contraction tiles
F32 = mybir.dt.float32
F32R = mybir.dt.float32r
NEG = -1.0e9

TRACE = False          # set by test harness to capture an NTFF profile
LAST_RESULT = None     # BassKernelResults from the most recent run

_cache = {}


def build_nc(debug=False):
    nc = bacc.Bacc()
    xt_d = nc.dram_tensor("xt", [C, T], F32R, kind="ExternalInput")
    wqk_d = nc.dram_tensor("wqk", [C, QKC], F32R, kind="ExternalInput")
    wv_d = nc.dram_tensor("wv", [C, GC], F32R, kind="ExternalInput")
    wp_d = nc.dram_tensor("wp", [GC, C], F32R, kind="ExternalInput")
    bqk_d = nc.dram_tensor("bqk", [128, QKC // 128], F32, kind="ExternalInput")
    mask_d = nc.dram_tensor("trimask", [128, 128], F32, kind="ExternalInput")
    ones_d = nc.dram_tensor("ones4", [128, HPG], F32R, kind="ExternalInput")
    out_d = nc.dram_tensor("out", [T, C], F32, kind="ExternalOutput")

    Exp = mybir.ActivationFunctionType.Exp

    with tile.TileContext(nc) as tc:
        with (
            tc.tile_pool(name="consts", bufs=1) as consts,
            tc.tile_pool(name="wpool", bufs=1) as wpool,
            tc.tile_pool(name="xt", bufs=1) as xtp,
            tc.tile_pool(name="qkt", bufs=1) as qktp,
            tc.tile_pool(name="v1", bufs=1) as v1p,
            tc.tile_pool(name="ytall", bufs=1) as ytallp,
            tc.tile_pool(name="expst", bufs=3) as expp,
            tc.tile_pool(name="att", bufs=1) as attp,
            tc.tile_pool(name="outsb", bufs=2) as outp,
            tc.tile_pool(name="mm", bufs=2, space="PSUM") as mmps,
            tc.tile_pool(name="s", bufs=2, space="PSUM") as sps,
            tc.tile_pool(name="yt", bufs=1, space="PSUM") as ytps,
        ):
            # ---- loads, in consumption order; x^T arrives by T-chunk so the
            # first qkT chains start after ~0.5 MB instead of ~10 MB ----
            bqk_sb = consts.tile([128, QKC // 128], F32, tag="bqk")
            nc.sync.dma_start(out=bqk_sb, in_=bqk_d[:, :])
            mask_sb = consts.tile([128, 128], F32, tag="mask")
            nc.sync.dma_start(out=mask_sb, in_=mask_d[:, :])
            xt_sb = [xtp.tile([128, T], F32R, tag=f"xt{k}", name=f"xt{k}") for k in range(NKC)]
            wqk_sb = [wpool.tile([128, QKC], F32R, tag=f"wqk{k}", name=f"wqk{k}") for k in range(NKC)]
            wv_sb = [wpool.tile([128, GC], F32R, tag=f"wv{k}", name=f"wv{k}") for k in range(NKC)]
            wp_sb = [wpool.tile([128, C], F32R, tag=f"wp{i}", name=f"wp{i}") for i in range(2)]
            for k in range(NKC):
                nc.sync.dma_start(out=wqk_sb[k], in_=wqk_d[k * 128:(k + 1) * 128, :])
                nc.sync.dma_start(
                    out=xt_sb[k][:, 0:TCH], in_=xt_d[k * 128:(k + 1) * 128, 0:TCH]
                )
            for ch in range(1, NQB):
                for k in range(NKC):
                    nc.sync.dma_start(
                        out=xt_sb[k][:, ch * TCH:(ch + 1) * TCH],
                        in_=xt_d[k * 128:(k + 1) * 128, ch * TCH:(ch + 1) * TCH],
                    )
                if ch == 1:
                    for k in range(NKC):
                        nc.sync.dma_start(out=wv_sb[k], in_=wv_d[k * 128:(k + 1) * 128, :])
            for i in range(2):
                nc.sync.dma_start(out=wp_sb[i], in_=wp_d[i * 128:(i + 1) * 128, :])

            qkt_sb = [qktp.tile([128, T], F32R, tag=f"qkt{ct}", name=f"qkt{ct}") for ct in range(4)]
            v1_sb = [v1p.tile([128, 65 * HPG], F32R, tag=f"v1_{t}", name=f"v1_{t}") for t in range(NT)]
            ytall_sb = [ytallp.tile([128, T], F32R, tag=f"yta{i}", name=f"yta{i}") for i in range(2)]

            # ---- work-unit builders (each returns a list of closures) ----
            def qkt_chains(ch):
                def mk(ct):
                    def go():
                        ps = mmps.tile([128, TCH], F32, tag="mm", name="mmq")
                        for k in range(NKC):
                            nc.tensor.matmul(
                                ps,
                                lhsT=wqk_sb[k][:, ct * 128:(ct + 1) * 128],
                                rhs=xt_sb[k][:, ch * TCH:(ch + 1) * TCH],
                                start=(k == 0),
                                stop=(k == NKC - 1),
                            )
                        nc.vector.tensor_scalar_add(
                            qkt_sb[ct][:, ch * TCH:(ch + 1) * TCH],
                            ps,
                            bqk_sb[:, ct:ct + 1],
                        )
                    return go
                return [mk(ct) for ct in range(4)]

            def v1_chains(ch):
                def mk(t):
                    def go():
                        ps = mmps.tile([128, GC], F32, tag="mm", name="mmv")
                        for k in range(NKC):
                            nc.tensor.matmul(
                                ps,
                                lhsT=xt_sb[k][:, t * 128:(t + 1) * 128],
                                rhs=wv_sb[k],
                                start=(k == 0),
                                stop=(k == NKC - 1),
                            )
                        v1t = v1_sb[t].rearrange("p (h c) -> p h c", c=65)
                        nc.sync.dma_start(out=v1t[:, :, 64:65], in_=ones_d[:, :, None])
                        src = ps.rearrange("p (h c) -> p h c", c=64)
                        nc.vector.tensor_copy(out=v1t[:, :, 0:64], in_=src)
                    return go
                return [mk(t) for t in range(4 * ch, 4 * ch + 4)]

            def attn_units(qb):
                """Units for query block qb: per (pair, kc-pair); the last
                unit of each pair appends the softmax normalization."""
                kc_max = 4 * qb + 4
                units = []
                state = {}

                def mk(pair, kcp):
                    def go(mid=None):
                        qtile = qkt_sb[pair]
                        ktile = qkt_sb[2 + pair]
                        if kcp == 0:
                            state[pair] = [
                                ytps.tile([65, TCH], F32, tag=f"yt{par}", name=f"yt{par}")
                                for par in range(2)
                            ]
                        yts = state[pair]
                        kcs = [2 * kcp, 2 * kcp + 1]
                        av_work = []
                        for par in range(2):
                            pr = par * 64
                            s = sps.tile([128, 2 * TCH], F32, tag="s", name="s")
                            c0s = []
                            for j, kc in enumerate(kcs):
                                di = kc - 4 * qb
                                c0 = max(di, 0) * 128
                                c0s.append(c0)
                                nc.tensor.matmul(
                                    s[:, j * TCH + c0:(j + 1) * TCH],
                                    lhsT=ktile[pr:pr + 64, kc * 128:(kc + 1) * 128],
                                    rhs=qtile[pr:pr + 64, qb * TCH + c0:(qb + 1) * TCH],
                                    start=True,
                                    stop=True,
                                    tile_position=(pr, 0),
                                )
                                if di >= 0:
                                    nc.vector.tensor_add(
                                        s[:, j * TCH + c0:j * TCH + c0 + 128],
                                        s[:, j * TCH + c0:j * TCH + c0 + 128],
                                        mask_sb,
                                    )
                            e = expp.tile([128, 2 * TCH], F32R, tag="e", name="e")
                            if c0s[1] > 0:
                                # diagonal pair: the region between the two
                                # halves' valid spans is uninitialized psum
                                for j in range(2):
                                    nc.scalar.activation(
                                        out=e[:, j * TCH + c0s[j]:(j + 1) * TCH],
                                        in_=s[:, j * TCH + c0s[j]:(j + 1) * TCH],
                                        func=Exp, scale=SCALE,
                                    )
                            else:
                                nc.scalar.activation(
                                    out=e[:, 0:2 * TCH], in_=s[:, 0:2 * TCH],
                                    func=Exp, scale=SCALE,
                                )
                            def av(par=par, e=e, c0s=tuple(c0s)):
                                for j, kc in enumerate(kcs):
                                    c0 = c0s[j]
                                    nc.tensor.matmul(
                                        yts[par][:, c0:TCH],
                                        lhsT=v1_sb[kc][:, (2 * pair + par) * 65:(2 * pair + par + 1) * 65],
                                        rhs=e[:, j * TCH + c0:(j + 1) * TCH],
                                        start=(kc == 0),
                                        stop=(kc == kc_max - 1),
                                    )
                            av_work.append(av)
                        # PE-dense filler lands here, covering the exp latency
                        if mid is not None:
                            mid()
                        for av in av_work:
                            av()
                        if 2 * kcp + 1 == kc_max - 1:
                            for par in range(2):
                                lsb = attp.tile([1, TCH], F32, tag="lsb", name="lsb")
                                nc.vector.tensor_copy(out=lsb, in_=yts[par][64:65, :])
                                rl = attp.tile([1, TCH], F32, tag="rl", name="rl")
                                nc.vector.reciprocal_approx_fast(out=rl, in_=lsb)
                                bc = attp.tile([64, TCH], F32, tag="bc", name="bc")
                                nc.gpsimd.partition_broadcast(out_ap=bc, in_ap=rl)
                                nc.vector.tensor_mul(
                                    ytall_sb[pair][par * 64:(par + 1) * 64,
                                                   qb * TCH:(qb + 1) * TCH],
                                    yts[par][0:64, :],
                                    bc,
                                )
                    return go

                for pair in range(2):
                    for kcp in range(kc_max // 2):
                        units.append(mk(pair, kcp))
                return units

            def proj_chains(qb):
                def mk(t, nch):
                    def go():
                        ps = mmps.tile([128, TCH], F32, tag="mm", name="mmp")
                        for pair in range(2):
                            nc.tensor.matmul(
                                ps,
                                lhsT=ytall_sb[pair][:, t * 128:(t + 1) * 128],
                                rhs=wp_sb[pair][:, nch * TCH:(nch + 1) * TCH],
                                start=(pair == 0),
                                stop=(pair == 1),
                            )
                        ot = outp.tile([128, TCH], F32, tag="ot", name="ot")
                        nc.vector.tensor_copy(out=ot, in_=ps)
                        nc.sync.dma_start(
                            out=out_d[t * 128:(t + 1) * 128, nch * TCH:(nch + 1) * TCH],
                            in_=ot,
                        )
                    return go
                return [mk(t, nch) for t in range(4 * qb, 4 * qb + 4) for nch in range(2)]

            def interleave(units, fillers):
                """Weave PE-dense filler chains into the attention units,
                placing them between each unit's S and AV matmul groups so
                the tensor engine stays busy while exp drains on ScalarE."""
                state = {"fi": 0}
                n = max(1, len(units))

                def advance(want):
                    while state["fi"] < min(want, len(fillers)):
                        fillers[state["fi"]]()
                        state["fi"] += 1

                for i, u in enumerate(units):
                    u(mid=lambda i=i: advance((2 * i + 1) * len(fillers) // (2 * n)))
                    advance((i + 1) * len(fillers) // n)
                advance(len(fillers))

            # ---- schedule ----
            for f in qkt_chains(0) + v1_chains(0):
                f()
            interleave(attn_units(0), qkt_chains(1) + v1_chains(1))
            interleave(attn_units(1), qkt_chains(2) + v1_chains(2))
            interleave(attn_units(2), qkt_chains(3) + v1_chains(3) + proj_chains(0))
            interleave(attn_units(3), proj_chains(1) + proj_chains(2))
            for f in proj_chains(3):
                f()

    nc.compile()
    return nc


def _tri_mask():
    i = np.arange(128)
    return np.where(i[:, None] <= i[None, :], 0.0, NEG).astype(np.float32)


def make_in_maps(x, W_attn, b_attn, W_proj):
    x = np.ascontiguousarray(np.asarray(x, dtype=np.float32))
    W_attn = np.asarray(W_attn, dtype=np.float32)
    b_attn = np.asarray(b_attn, dtype=np.float32)
    W_proj = np.asarray(W_proj, dtype=np.float32)
    mask = _tri_mask()
    xts = [np.ascontiguousarray(x[b].T) for b in range(B)]
    in_maps = []
    for c in range(NCORES):
        b, g = divmod(c, NGROUPS)
        qs = slice(g * GC, (g + 1) * GC)
        ks = slice(C + g * GC, C + (g + 1) * GC)
        vs = slice(2 * C + g * GC, 2 * C + (g + 1) * GC)
        wqk = np.ascontiguousarray(
            np.concatenate([W_attn[:, qs], W_attn[:, ks]], axis=1)
        )
        bqk = np.ascontiguousarray(
            np.concatenate([b_attn[qs], b_attn[ks]]).reshape(QKC // 128, 128).T
        )
        in_maps.append(
            {
                "xt": xts[b],
                "wqk": wqk,
                "wv": np.ascontiguousarray(W_attn[:, vs]),
                "wp": np.ascontiguousarray(W_proj[g * GC:(g + 1) * GC, :]),
                "bqk": bqk,
                "trimask": mask,
                "ones4": np.ones((128, HPG), dtype=np.float32),
            }
        )
    return in_maps


def kernel(x, W_attn, b_attn, W_proj, b_proj):
    global LAST_RESULT
    if "nc" not in _cache:
        _cache["nc"] = build_nc()
    nc = _cache["nc"]
    in_maps = make_in_maps(x, W_attn, b_attn, W_proj)
    res = run_bass_kernel_spmd(
        nc, in_maps, core_ids=list(range(NCORES)), trace=TRACE
    )
    LAST_RESULT = res
    out = np.zeros((B, T, C), dtype=np.float32)
    for c in range(NCORES):
        out[c // NGROUPS] += res.results[c]["out"]
    # v-bias and proj-bias correction (softmax rows sum to 1)
    b_attn = np.asarray(b_attn, dtype=np.float32)
    W_proj = np.asarray(W_proj, dtype=np.float32)
    b_proj = np.asarray(b_proj, dtype=np.float32)
    out += b_proj + np.asarray(b_attn)[2 * C:] @ W_proj
    return out


# revision 18
# speedup vs baseline: 1.0272x; 1.0272x over previous
"""Causal self-attention Trainium2 kernel (Bass/Tile, SPMD over 8 NeuronCores).

Problem: B=2, T=2048, C=1024, H=16 heads, HD=64; out = proj(softmax(mask(q k^T * s)) v).

Sharding (Megatron-style): core c handles batch b=c//4 and head group g=c%4
(4 heads).  Each core computes qkv for its heads, causal attention, and a
partial projection out_partial = y_local @ W_proj[rows of g].  Host sums the 4
partials per batch; q/k biases are applied on-device, while the v/proj biases
are folded into a host-side constant correction (softmax rows sum to 1, so
attn @ (v + bv) = attn @ v + bv).

Device layout notes:
  - Host passes x^T so the C-contraction dim lands on SBUF partitions.
  - q,k are produced transposed (qT/kT: [head cols, T]); scores are computed
    transposed (S^T: [keys, queries]) so the AV matmul needs no transposes.
  - v gets an appended ones-column; the AV matmul then yields softmax row sums
    (l) in output row 64, normalized via reciprocal + partition-broadcast.
  - All matmuls run in float32r (~1.6e-4 matmul rel err, 4x faster than fp32).
  - Scores for two key-tiles land in one 2-bank PSUM tensor and share one
    wide exp ACTIVATE (amortizes the scalar engine's per-op overhead).
  - Emission interleaves attention units with qkv/proj matmul chains at fine
    grain so the tensor engine never idles long enough for the HAM clock
    gate to re-throttle it to 1.2 GHz while exp drains on the scalar engine.
"""

import numpy as np

import concourse.bacc as bacc
import concourse.mybir as mybir
import concourse.tile as tile
from concourse.bass_utils import run_bass_kernel_spmd

B, T, C, H, HD = 2, 2048, 1024, 16, 64
SCALE = 0.1 / (HD**0.5)
NCORES = 8
NGROUPS = 4           # head groups (tensor-parallel dim)
HPG = H // NGROUPS    # 4 heads per core
GC = HPG * HD         # 256 head-cols per core
QKC = 2 * GC          # 512 q+k cols per core
TCH = 512             # psum free-dim chunk
NT = T // 128         # 16 row tiles
NQB = T // TCH        # 4 query blocks
NKC = 8               # C/128 contraction tiles
F32 = mybir.dt.float32
F32R = mybir.dt.float32r
NEG = -1.0e9

TRACE = False          # set by test harness to capture an NTFF profile
LAST_RESULT = None     # BassKernelResults from the most recent run

_cache = {}


def build_nc(debug=False):
    nc = bacc.Bacc()
    xt_d = nc.dram_tensor("xt", [C, T], F32R, kind="ExternalInput")
    wqk_d = nc.dram_tensor("wqk", [C, QKC], F32R, kind="ExternalInput")
    wv_d = nc.dram_tensor("wv", [C, GC], F32R, kind="ExternalInput")
    wp_d = nc.dram_tensor("wp", [GC, C], F32R, kind="ExternalInput")
    bqk_d = nc.dram_tensor("bqk", [128, QKC // 128], F32, kind="ExternalInput")
    mask_d = nc.dram_tensor("trimask", [128, 128], F32, kind="ExternalInput")
    ones_d = nc.dram_tensor("ones4", [128, HPG], F32R, kind="ExternalInput")
    out_d = nc.dram_tensor("out", [T, C], F32, kind="ExternalOutput")

    Exp = mybir.ActivationFunctionType.Exp

    with tile.TileContext(nc) as tc:
        with (
            tc.tile_pool(name="consts", bufs=1) as consts,
            tc.tile_pool(name="wpool", bufs=1) as wpool,
            tc.tile_pool(name="xt", bufs=1) as xtp,
            tc.tile_pool(name="qkt", bufs=1) as qktp,
            tc.tile_pool(name="v1", bufs=1) as v1p,
            tc.tile_pool(name="ytall", bufs=1) as ytallp,
            tc.tile_pool(name="expst", bufs=3) as expp,
            tc.tile_pool(name="att", bufs=1) as attp,
            tc.tile_pool(name="outsb", bufs=2) as outp,
            tc.tile_pool(name="mm", bufs=2, space="PSUM") as mmps,
            tc.tile_pool(name="s", bufs=2, space="PSUM") as sps,
            tc.tile_pool(name="yt", bufs=1, space="PSUM") as ytps,
        ):
            # ---- loads, in consumption order; x^T arrives by T-chunk so the
            # first qkT chains start after ~0.5 MB instead of ~10 MB ----
            bqk_sb = consts.tile([128, QKC // 128], F32, tag="bqk")
            nc.sync.dma_start(out=bqk_sb, in_=bqk_d[:, :])
            mask_sb = consts.tile([128, 128], F32, tag="mask")
            nc.sync.dma_start(out=mask_sb, in_=mask_d[:, :])
            xt_sb = [xtp.tile([128, T], F32R, tag=f"xt{k}", name=f"xt{k}") for k in range(NKC)]
            wqk_sb = [wpool.tile([128, QKC], F32R, tag=f"wqk{k}", name=f"wqk{k}") for k in range(NKC)]
            wv_sb = [wpool.tile([128, GC], F32R, tag=f"wv{k}", name=f"wv{k}") for k in range(NKC)]
            wp_sb = [wpool.tile([128, C], F32R, tag=f"wp{i}", name=f"wp{i}") for i in range(2)]
            qengines = [nc.sync, nc.gpsimd, nc.scalar]
            for k in range(NKC):
                qengines[k % 3].dma_start(out=wqk_sb[k], in_=wqk_d[k * 128:(k + 1) * 128, :])
                qengines[(k + 1) % 3].dma_start(
                    out=xt_sb[k][:, 0:TCH], in_=xt_d[k * 128:(k + 1) * 128, 0:TCH]
                )
            for ch in range(1, NQB):
                for k in range(NKC):
                    nc.sync.dma_start(
                        out=xt_sb[k][:, ch * TCH:(ch + 1) * TCH],
                        in_=xt_d[k * 128:(k + 1) * 128, ch * TCH:(ch + 1) * TCH],
                    )
                if ch == 1:
                    for k in range(NKC):
                        nc.sync.dma_start(out=wv_sb[k], in_=wv_d[k * 128:(k + 1) * 128, :])
            for i in range(2):
                nc.sync.dma_start(out=wp_sb[i], in_=wp_d[i * 128:(i + 1) * 128, :])

            qkt_sb = [qktp.tile([128, T], F32R, tag=f"qkt{ct}", name=f"qkt{ct}") for ct in range(4)]
            v1_sb = [v1p.tile([128, 65 * HPG], F32R, tag=f"v1_{t}", name=f"v1_{t}") for t in range(NT)]
            ytall_sb = [ytallp.tile([128, T], F32R, tag=f"yta{i}", name=f"yta{i}") for i in range(2)]

            # ---- work-unit builders (each returns a list of closures) ----
            def qkt_chains(ch):
                def mk(ct):
                    def go():
                        ps = mmps.tile([128, TCH], F32, tag="mm", name="mmq")
                        for k in range(NKC):
                            nc.tensor.matmul(
                                ps,
                                lhsT=wqk_sb[k][:, ct * 128:(ct + 1) * 128],
                                rhs=xt_sb[k][:, ch * TCH:(ch + 1) * TCH],
                                start=(k == 0),
                                stop=(k == NKC - 1),
                            )
                        nc.vector.tensor_scalar_add(
                            qkt_sb[ct][:, ch * TCH:(ch + 1) * TCH],
                            ps,
                            bqk_sb[:, ct:ct + 1],
                        )
                    return go
                return [mk(ct) for ct in range(4)]

            def v1_chains(ch):
                def mk(t):
                    def go():
                        ps = mmps.tile([128, GC], F32, tag="mm", name="mmv")
                        for k in range(NKC):
                            nc.tensor.matmul(
                                ps,
                                lhsT=xt_sb[k][:, t * 128:(t + 1) * 128],
                                rhs=wv_sb[k],
                                start=(k == 0),
                                stop=(k == NKC - 1),
                            )
                        v1t = v1_sb[t].rearrange("p (h c) -> p h c", c=65)
                        nc.gpsimd.dma_start(out=v1t[:, :, 64:65], in_=ones_d[:, :, None])
                        src = ps.rearrange("p (h c) -> p h c", c=64)
                        nc.vector.tensor_copy(out=v1t[:, :, 0:64], in_=src)
                    return go
                return [mk(t) for t in range(4 * ch, 4 * ch + 4)]

            def attn_units(qb):
                """Units for query block qb: per (pair, kc-pair); the last
                unit of each pair appends the softmax normalization."""
                kc_max = 4 * qb + 4
                units = []
                state = {}

                def mk(pair, kcp):
                    def go(mid=None):
                        qtile = qkt_sb[pair]
                        ktile = qkt_sb[2 + pair]
                        if kcp == 0:
                            state[pair] = [
                                ytps.tile([65, TCH], F32, tag=f"yt{par}", name=f"yt{par}")
                                for par in range(2)
                            ]
                        yts = state[pair]
                        kcs = [2 * kcp, 2 * kcp + 1]
                        av_work = []
                        for par in range(2):
                            pr = par * 64
                            s = sps.tile([128, 2 * TCH], F32, tag="s", name="s")
                            c0s = []
                            for j, kc in enumerate(kcs):
                                di = kc - 4 * qb
                                c0 = max(di, 0) * 128
                                c0s.append(c0)
                                nc.tensor.matmul(
                                    s[:, j * TCH + c0:(j + 1) * TCH],
                                    lhsT=ktile[pr:pr + 64, kc * 128:(kc + 1) * 128],
                                    rhs=qtile[pr:pr + 64, qb * TCH + c0:(qb + 1) * TCH],
                                    start=True,
                                    stop=True,
                                    tile_position=(pr, 0),
                                )
                                if di >= 0:
                                    nc.vector.tensor_add(
                                        s[:, j * TCH + c0:j * TCH + c0 + 128],
                                        s[:, j * TCH + c0:j * TCH + c0 + 128],
                                        mask_sb,
                                    )
                            e = expp.tile([128, 2 * TCH], F32R, tag="e", name="e")
                            if c0s[1] > 0:
                                # diagonal pair: the region between the two
                                # halves' valid spans is uninitialized psum
                                for j in range(2):
                                    nc.scalar.activation(
                                        out=e[:, j * TCH + c0s[j]:(j + 1) * TCH],
                                        in_=s[:, j * TCH + c0s[j]:(j + 1) * TCH],
                                        func=Exp, scale=SCALE,
                                    )
                            else:
                                nc.scalar.activation(
                                    out=e[:, 0:2 * TCH], in_=s[:, 0:2 * TCH],
                                    func=Exp, scale=SCALE,
                                )
                            def av(par=par, e=e, c0s=tuple(c0s)):
                                for j, kc in enumerate(kcs):
                                    c0 = c0s[j]
                                    nc.tensor.matmul(
                                        yts[par][:, c0:TCH],
                                        lhsT=v1_sb[kc][:, (2 * pair + par) * 65:(2 * pair + par + 1) * 65],
                                        rhs=e[:, j * TCH + c0:(j + 1) * TCH],
                                        start=(kc == 0),
                                        stop=(kc == kc_max - 1),
                                    )
                            av_work.append(av)
                        # PE-dense filler lands here, covering the exp latency
                        if mid is not None:
                            mid()
                        for av in av_work:
                            av()
                        if 2 * kcp + 1 == kc_max - 1:
                            for par in range(2):
                                lsb = attp.tile([1, TCH], F32, tag="lsb", name="lsb")
                                nc.vector.tensor_copy(out=lsb, in_=yts[par][64:65, :])
                                rl = attp.tile([1, TCH], F32, tag="rl", name="rl")
                                nc.vector.reciprocal_approx_fast(out=rl, in_=lsb)
                                bc = attp.tile([64, TCH], F32, tag="bc", name="bc")
                                nc.gpsimd.partition_broadcast(out_ap=bc, in_ap=rl)
                                nc.vector.tensor_mul(
                                    ytall_sb[pair][par * 64:(par + 1) * 64,
                                                   qb * TCH:(qb + 1) * TCH],
                                    yts[par][0:64, :],
                                    bc,
                                )
                    return go

                for pair in range(2):
                    for kcp in range(kc_max // 2):
                        units.append(mk(pair, kcp))
                return units

            def proj_chains(qb):
                def mk(t, nch):
                    def go():
                        ps = mmps.tile([128, TCH], F32, tag="mm", name="mmp")
                        for pair in range(2):
                            nc.tensor.matmul(
                                ps,
                                lhsT=ytall_sb[pair][:, t * 128:(t + 1) * 128],
                                rhs=wp_sb[pair][:, nch * TCH:(nch + 1) * TCH],
                                start=(pair == 0),
                                stop=(pair == 1),
                            )
                        ot = outp.tile([128, TCH], F32, tag="ot", name="ot")
                        nc.vector.tensor_copy(out=ot, in_=ps)
                        nc.sync.dma_start(
                            out=out_d[t * 128:(t + 1) * 128, nch * TCH:(nch + 1) * TCH],
                            in_=ot,
                        )
                    return go
                return [mk(t, nch) for t in range(4 * qb, 4 * qb + 4) for nch in range(2)]

            def dummy_chains(n):
                def mk(i):
                    def go():
                        ps = mmps.tile([128, TCH], F32, tag="mm", name="mmd")
                        for j in range(2):
                            nc.tensor.matmul(
                                ps,
                                lhsT=wqk_sb[0][:, 0:128],
                                rhs=xt_sb[0][:, 0:TCH],
                                start=(j == 0),
                                stop=(j == 1),
                            )
                    return go
                return [mk(i) for i in range(n)]

            def interleave(units, fillers):
                """Weave PE-dense filler chains into the attention units,
                placing them between each unit's S and AV matmul groups so
                the tensor engine stays busy while exp drains on ScalarE."""
                state = {"fi": 0}
                n = max(1, len(units))

                def advance(want):
                    while state["fi"] < min(want, len(fillers)):
                        fillers[state["fi"]]()
                        state["fi"] += 1

                for i, u in enumerate(units):
                    u(mid=lambda i=i: advance((2 * i + 1) * len(fillers) // (2 * n)))
                    advance((i + 1) * len(fillers) // n)
                advance(len(fillers))

            # ---- schedule ----
            for f in qkt_chains(0) + v1_chains(0):
                f()
            interleave(attn_units(0), qkt_chains(1) + v1_chains(1))
            interleave(attn_units(1), qkt_chains(2) + v1_chains(2))
            interleave(attn_units(2), qkt_chains(3) + proj_chains(0) + dummy_chains(4))
            interleave(attn_units(3),
                       v1_chains(3) + proj_chains(1) + proj_chains(2) + dummy_chains(10))
            for f in proj_chains(3):
                f()

    nc.compile()
    return nc


def _tri_mask():
    i = np.arange(128)
    return np.where(i[:, None] <= i[None, :], 0.0, NEG).astype(np.float32)


def make_in_maps(x, W_attn, b_attn, W_proj):
    x = np.ascontiguousarray(np.asarray(x, dtype=np.float32))
    W_attn = np.asarray(W_attn, dtype=np.float32)
    b_attn = np.asarray(b_attn, dtype=np.float32)
    W_proj = np.asarray(W_proj, dtype=np.float32)
    mask = _tri_mask()
    xts = [np.ascontiguousarray(x[b].T) for b in range(B)]
    in_maps = []
    for c in range(NCORES):
        b, g = divmod(c, NGROUPS)
        qs = slice(g * GC, (g + 1) * GC)
        ks = slice(C + g * GC, C + (g + 1) * GC)
        vs = slice(2 * C + g * GC, 2 * C + (g + 1) * GC)
        wqk = np.ascontiguousarray(
            np.concatenate([W_attn[:, qs], W_attn[:, ks]], axis=1)
        )
        bqk = np.ascontiguousarray(
            np.concatenate([b_attn[qs], b_attn[ks]]).reshape(QKC // 128, 128).T
        )
        in_maps.append(
            {
                "xt": xts[b],
                "wqk": wqk,
                "wv": np.ascontiguousarray(W_attn[:, vs]),
                "wp": np.ascontiguousarray(W_proj[g * GC:(g + 1) * GC, :]),
                "bqk": bqk,
                "trimask": mask,
                "ones4": np.ones((128, HPG), dtype=np.float32),
            }
        )
    return in_maps


def kernel(x, W_attn, b_attn, W_proj, b_proj):
    global LAST_RESULT
    if "nc" not in _cache:
        _cache["nc"] = build_nc()
    nc = _cache["nc"]
    in_maps = make_in_maps(x, W_attn, b_attn, W_proj)
    res = run_bass_kernel_spmd(
        nc, in_maps, core_ids=list(range(NCORES)), trace=TRACE
    )
    LAST_RESULT = res
    out = np.zeros((B, T, C), dtype=np.float32)
    for c in range(NCORES):
        out[c // NGROUPS] += res.results[c]["out"]
    # v-bias and proj-bias correction (softmax rows sum to 1)
    b_attn = np.asarray(b_attn, dtype=np.float32)
    W_proj = np.asarray(W_proj, dtype=np.float32)
    b_proj = np.asarray(b_proj, dtype=np.float32)
    out += b_proj + np.asarray(b_attn)[2 * C:] @ W_proj
    return out
